# revision 2
# baseline (speedup 1.0000x reference)
"""GNN message-passing kernel for Trainium2 (8 NeuronCores, data-parallel).

Computes msg = vs @ W + b.sum(0) for vs [2M, 8] f32, W/b [8, 64] f32.

Strategy:
  - Shard vs rows 8 ways (250k rows/core); W/b replicated (no gradients here,
    forward only).
  - Precision: fp16 hi/lo split, 3 cross terms (hi*hi, lo*hi, hi*lo) gives
    fp32-grade accuracy while running the PE at 1 cycle/row (fp32 matmul is
    4 cycles/row).
  - Per 640-node chunk: the host-packed [B, 24] fp16 tensor is DMA'd in a
    (p (c t s)) layout, PE-transposed to put the 24-value groups on
    partitions (K=120 = 5 t-blocks of 24), then one matmul against a
    block-diagonal [120, 320] weight matrix produces out[p, 64t+h] =
    msg[node(p, t), h] — giving fully contiguous per-partition output DMA.
  - Bias is folded into the PSUM->SBUF evacuation (DVE tensor_add).
"""

import numpy as np
import concourse.bacc as bacc
import concourse.mybir as mybir
from concourse.tile import TileContext
from concourse.masks import make_identity
from concourse.bass_utils import run_bass_kernel_spmd

F32 = mybir.dt.float32
F16 = mybir.dt.float16

B = 2_000_000
NCORES = 8
NS = B // NCORES          # 250_000 nodes per core
TB = 5                    # t-blocks per matmul
CHUNK = 128 * TB          # 640 nodes per chunk
GC_MAIN = 8               # chunks per tile in the main loop

# Tile plan per core: 48 tiles x 8 chunks + 1 tile x 6 chunks = 249_600 nodes,
# plus one single-chunk tile re-covering the final 640 nodes (overlap region
# is written twice with identical values).
TILES = [(i * CHUNK * GC_MAIN, GC_MAIN) for i in range(48)]
TILES.append((48 * CHUNK * GC_MAIN, 6))
TILES.append((NS - CHUNK, 1))

_nc_cache = None


def _build():
    nc = bacc.Bacc()
    p24 = nc.dram_tensor("p24", [NS, 24], F16, kind="ExternalInput")
    ws = nc.dram_tensor("ws", [120, 320], F16, kind="ExternalInput")
    bias = nc.dram_tensor("bias", [128, 320], F32, kind="ExternalInput")
    out = nc.dram_tensor("out", [NS, 64], F32, kind="ExternalOutput")

    with TileContext(nc) as tc:
        with (
            tc.tile_pool(name="const", bufs=1) as cpool,
            tc.tile_pool(name="inp", bufs=3) as in_pool,
            tc.tile_pool(name="tsb", bufs=4) as t_pool,
            tc.tile_pool(name="outp", bufs=3) as out_pool,
            tc.tile_pool(name="psum", bufs=3, space="PSUM") as psum_pool,
        ):
            ident = cpool.tile([128, 128], F16)
            make_identity(nc, ident[:])
            ws_sb = cpool.tile([120, 320], F16)
            nc.sync.dma_start(out=ws_sb[:], in_=ws[:])
            bias_sb = cpool.tile([128, 320], F32)
            nc.sync.dma_start(out=bias_sb[:], in_=bias[:])

            for base, gc in TILES:
                in_tile = in_pool.tile([128, 120 * GC_MAIN], F16, tag="in")
                nc.sync.dma_start(
                    out=in_tile[:, : 120 * gc],
                    in_=p24[base : base + CHUNK * gc, :].rearrange(
                        "(p c t) s -> p (c t s)", p=128, c=gc, t=TB
                    ),
                )
                out_sb = out_pool.tile([128, 320 * GC_MAIN], F32, tag="out")
                for c in range(gc):
                    t_psum = psum_pool.tile([120, 128], F16, tag="tp")
                    nc.tensor.transpose(
                        t_psum[:], in_tile[:, 120 * c : 120 * c + 120], ident[:]
                    )
                    t_sb = t_pool.tile([120, 128], F16, tag="t")
                    nc.scalar.copy(out=t_sb[:], in_=t_psum[:])
                    mm_psum = psum_pool.tile([128, 320], F32, tag="mm")
                    nc.tensor.matmul(
                        mm_psum[:], t_sb[:], ws_sb[:], start=True, stop=True
                    )
                    nc.vector.tensor_add(
                        out=out_sb[:, 320 * c : 320 * c + 320],
                        in0=mm_psum[:],
                        in1=bias_sb[:],
                    )
                nc.sync.dma_start(
                    out=out[base : base + CHUNK * gc, :].rearrange(
                        "(p c t) h -> p (c t h)", p=128, c=gc, t=TB
                    ),
                    in_=out_sb[:, : 320 * gc],
                )
    nc.compile()
    return nc


def _get_nc():
    global _nc_cache
    if _nc_cache is None:
        _nc_cache = _build()
    return _nc_cache


def _pack24(vs: np.ndarray) -> np.ndarray:
    hi = vs.astype(np.float16)
    lo = (vs - hi.astype(np.float32)).astype(np.float16)
    p = np.empty((vs.shape[0], 24), dtype=np.float16)
    p[:, 0::3] = hi
    p[:, 1::3] = lo
    p[:, 2::3] = hi
    return p


def _make_ws(W: np.ndarray) -> np.ndarray:
    w_hi = W.astype(np.float16)
    w_lo = (W - w_hi.astype(np.float32)).astype(np.float16)
    ws = np.zeros((120, 320), dtype=np.float16)
    for t in range(TB):
        for i in range(8):
            ws[24 * t + 3 * i + 0, 64 * t : 64 * t + 64] = w_hi[i]
            ws[24 * t + 3 * i + 1, 64 * t : 64 * t + 64] = w_hi[i]
            ws[24 * t + 3 * i + 2, 64 * t : 64 * t + 64] = w_lo[i]
    return ws


def kernel(vs: np.ndarray, W: np.ndarray, b: np.ndarray, _trace=False):
    vs = np.asarray(vs, dtype=np.float32)
    W = np.asarray(W, dtype=np.float32)
    b = np.asarray(b, dtype=np.float32)

    nc = _get_nc()

    ws = _make_ws(W)
    bsum = b.sum(axis=0, dtype=np.float32)
    bias = np.broadcast_to(np.tile(bsum, TB), (128, 320)).copy()

    p24 = _pack24(vs.reshape(B, 8))
    in_maps = [
        {"p24": np.ascontiguousarray(p24[k * NS : (k + 1) * NS]), "ws": ws,
         "bias": bias}
        for k in range(NCORES)
    ]

    res = run_bass_kernel_spmd(nc, in_maps, core_ids=list(range(NCORES)))
    out = np.concatenate([r["out"] for r in res.results], axis=0)
    if _trace:
        kernel.last_result = res
    return out


# revision 10
# speedup vs baseline: 1.1671x; 1.1671x over previous
"""GNN message-passing kernel for Trainium2 (8 NeuronCores, data-parallel).

Computes msg = vs @ W + b.sum(0) for vs [2M, 8] f32, W/b [8, 64] f32.

Strategy:
  - Shard vs rows 8 ways (250k rows/core); W/b replicated (no gradients here,
    forward only).
  - Precision: fp16 hi/lo split, 3 cross terms (hi*hi, lo*hi, hi*lo) gives
    fp32-grade accuracy while running the PE at 1 cycle/row (fp32 matmul is
    4 cycles/row).
  - Per 640-node chunk: the host-packed [B, 24] fp16 tensor is DMA'd in a
    (p (c t s)) layout, PE-transposed to put the 24-value groups on
    partitions (K=120 = 5 t-blocks of 24), then one matmul against a
    block-diagonal [120, 320] weight matrix produces out[p, 64t+h] =
    msg[node(p, t), h] — giving fully contiguous per-partition output DMA.
  - Bias is folded into the PSUM->SBUF evacuation (DVE tensor_add).
"""

import numpy as np
import concourse.bacc as bacc
import concourse.mybir as mybir
from concourse.tile import TileContext
from concourse.masks import make_identity
from concourse.bass_utils import run_bass_kernel_spmd

F32 = mybir.dt.float32
F16 = mybir.dt.float16

B = 2_000_000
NCORES = 8
NS = B // NCORES          # 250_000 nodes per core
TB = 5                    # t-blocks per matmul
CHUNK = 128 * TB          # 640 nodes per chunk
GC_MAIN = 16              # chunks per tile in the main loop


def _tile_plan(gc_main):
    # Full chunks per core: 390 regular + 1 overlap tile re-covering the
    # final 640 nodes (overlap region written twice with identical values).
    n_full = 390
    tiles = [(i * CHUNK * gc_main, gc_main) for i in range(n_full // gc_main)]
    rem = n_full % gc_main
    if rem:
        tiles.append(((n_full - rem) * CHUNK, rem))
    tiles.append((NS - CHUNK, 1))
    return tiles


_nc_cache = None


def _build(gc_main=GC_MAIN, bufs_in=5, bufs_t=8, bufs_out=5, bufs_ptp=2,
           bufs_pmm=3):
    nc = bacc.Bacc()
    p24 = nc.dram_tensor("p24", [NS, 24], F16, kind="ExternalInput")
    ws = nc.dram_tensor("ws", [120, 320], F16, kind="ExternalInput")
    bias = nc.dram_tensor("bias", [128, 640], F32, kind="ExternalInput")
    out = nc.dram_tensor("out", [NS, 64], F32, kind="ExternalOutput")

    with TileContext(nc) as tc:
        with (
            tc.tile_pool(name="const", bufs=1) as cpool,
            tc.tile_pool(name="inp", bufs=bufs_in) as in_pool,
            tc.tile_pool(name="tsb", bufs=bufs_t) as t_pool,
            tc.tile_pool(name="outp", bufs=bufs_out) as out_pool,
            tc.tile_pool(name="ptp", bufs=bufs_ptp, space="PSUM") as ptp_pool,
            tc.tile_pool(name="pmm", bufs=bufs_pmm, space="PSUM") as pmm_pool,
        ):
            ident = cpool.tile([128, 128], F16)
            make_identity(nc, ident[:])
            ws_sb = cpool.tile([120, 320], F16)
            nc.sync.dma_start(out=ws_sb[:], in_=ws[:])
            bias2_sb = cpool.tile([128, 640], F32)
            nc.sync.dma_start(out=bias2_sb[:], in_=bias[:])
            bias_sb = bias2_sb[:, :320]

            for base, gc in _tile_plan(gc_main):
                in_tile = in_pool.tile([128, 120 * gc_main], F16, tag="in")
                nc.sync.dma_start(
                    out=in_tile[:, : 120 * gc],
                    in_=p24[base : base + CHUNK * gc, :].rearrange(
                        "(p c t) s -> p (c t s)", p=128, c=gc, t=TB
                    ),
                )
                out_sb = out_pool.tile([128, 320 * gc_main], F32, tag="out")
                # Chunks processed in pairs: two transposes share one PSUM
                # tile / one ACT copy; two matmuls land in one two-bank PSUM
                # tile (each within its own bank) so one DVE tensor_add
                # evacuates + biases both.
                for c0 in range(0, gc, 2):
                    pair = min(2, gc - c0)
                    t_psum = ptp_pool.tile([120, 256], F16, tag="tp")
                    for k in range(pair):
                        nc.tensor.transpose(
                            t_psum[:, 128 * k : 128 * k + 128],
                            in_tile[:, 120 * (c0 + k) : 120 * (c0 + k) + 120],
                            ident[:],
                        )
                    t_sb = t_pool.tile([120, 256], F16, tag="t")
                    nc.scalar.copy(
                        out=t_sb[:, : 128 * pair], in_=t_psum[:, : 128 * pair]
                    )
                    mm_psum = pmm_pool.tile([128, 1024], F32, tag="mm")
                    for k in range(pair):
                        nc.tensor.matmul(
                            mm_psum[:, 512 * k : 512 * k + 320],
                            t_sb[:, 128 * k : 128 * k + 128],
                            ws_sb[:],
                            start=True,
                            stop=True,
                        )
                    if pair == 2:
                        src = mm_psum[:].rearrange("p (k n) -> p k n", k=2)[:, :, :320]
                        nc.vector.tensor_add(
                            out=out_sb[:, 320 * c0 : 320 * c0 + 640].rearrange(
                                "p (k n) -> p k n", k=2
                            ),
                            in0=src,
                            in1=bias2_sb[:].rearrange("p (k n) -> p k n", k=2)[
                                :, :, :320
                            ],
                        )
                    else:
                        nc.vector.tensor_add(
                            out=out_sb[:, 320 * c0 : 320 * c0 + 320],
                            in0=mm_psum[:, :320],
                            in1=bias_sb,
                        )
                nc.sync.dma_start(
                    out=out[base : base + CHUNK * gc, :].rearrange(
                        "(p c t) h -> p (c t h)", p=128, c=gc, t=TB
                    ),
                    in_=out_sb[:, : 320 * gc],
                )
    nc.compile()
    return nc


def _get_nc():
    global _nc_cache
    if _nc_cache is None:
        _nc_cache = _build()
    return _nc_cache


def _pack24(vs: np.ndarray) -> np.ndarray:
    hi = vs.astype(np.float16)
    lo = (vs - hi.astype(np.float32)).astype(np.float16)
    p = np.empty((vs.shape[0], 24), dtype=np.float16)
    p[:, 0::3] = hi
    p[:, 1::3] = lo
    p[:, 2::3] = hi
    return p


def _make_ws(W: np.ndarray) -> np.ndarray:
    w_hi = W.astype(np.float16)
    w_lo = (W - w_hi.astype(np.float32)).astype(np.float16)
    ws = np.zeros((120, 320), dtype=np.float16)
    for t in range(TB):
        for i in range(8):
            ws[24 * t + 3 * i + 0, 64 * t : 64 * t + 64] = w_hi[i]
            ws[24 * t + 3 * i + 1, 64 * t : 64 * t + 64] = w_hi[i]
            ws[24 * t + 3 * i + 2, 64 * t : 64 * t + 64] = w_lo[i]
    return ws


def kernel(vs: np.ndarray, W: np.ndarray, b: np.ndarray, _trace=False):
    vs = np.asarray(vs, dtype=np.float32)
    W = np.asarray(W, dtype=np.float32)
    b = np.asarray(b, dtype=np.float32)

    nc = _get_nc()

    ws = _make_ws(W)
    bsum = b.sum(axis=0, dtype=np.float32)
    bias = np.broadcast_to(np.tile(bsum, 2 * TB), (128, 640)).copy()

    p24 = _pack24(vs.reshape(B, 8))
    in_maps = [
        {"p24": np.ascontiguousarray(p24[k * NS : (k + 1) * NS]), "ws": ws,
         "bias": bias}
        for k in range(NCORES)
    ]

    res = run_bass_kernel_spmd(nc, in_maps, core_ids=list(range(NCORES)))
    out = np.concatenate([r["out"] for r in res.results], axis=0)
    if _trace:
        kernel.last_result = res
    return out


# revision 11
# speedup vs baseline: 1.1760x; 1.0076x over previous
"""GNN message-passing kernel for Trainium2 (8 NeuronCores, data-parallel).

Computes msg = vs @ W + b.sum(0) for vs [2M, 8] f32, W/b [8, 64] f32.

Strategy:
  - Shard vs rows 8 ways (250k rows/core); W/b replicated (no gradients here,
    forward only).
  - Precision: fp16 hi/lo split, 3 cross terms (hi*hi, lo*hi, hi*lo) gives
    fp32-grade accuracy while running the PE at 1 cycle/row (fp32 matmul is
    4 cycles/row).
  - Per 640-node chunk: the host-packed [B, 24] fp16 tensor is DMA'd in a
    (p (c t s)) layout, PE-transposed to put the 24-value groups on
    partitions (K=120 = 5 t-blocks of 24), then one matmul against a
    block-diagonal [120, 320] weight matrix produces out[p, 64t+h] =
    msg[node(p, t), h] — giving fully contiguous per-partition output DMA.
  - Bias is folded into the PSUM->SBUF evacuation (DVE tensor_add).
"""

import numpy as np
import concourse.bacc as bacc
import concourse.mybir as mybir
from concourse.tile import TileContext
from concourse.masks import make_identity
from concourse.bass_utils import run_bass_kernel_spmd

F32 = mybir.dt.float32
F16 = mybir.dt.float16

B = 2_000_000
NCORES = 8
NS = B // NCORES          # 250_000 nodes per core
TB = 5                    # t-blocks per matmul
CHUNK = 128 * TB          # 640 nodes per chunk
GC_MAIN = 16              # chunks per tile in the main loop


def _tile_plan(gc_main):
    # Full chunks per core: 390 regular + 1 overlap tile re-covering the
    # final 640 nodes (overlap region written twice with identical values).
    n_full = 390
    tiles = [(i * CHUNK * gc_main, gc_main) for i in range(n_full // gc_main)]
    rem = n_full % gc_main
    if rem:
        tiles.append(((n_full - rem) * CHUNK, rem))
    tiles.append((NS - CHUNK, 1))
    return tiles


_nc_cache = None


def _build(gc_main=GC_MAIN, bufs_in=6, bufs_t=8, bufs_out=6, bufs_ptp=2,
           bufs_pmm=3):
    nc = bacc.Bacc()
    p24 = nc.dram_tensor("p24", [NS, 24], F16, kind="ExternalInput")
    ws = nc.dram_tensor("ws", [120, 320], F16, kind="ExternalInput")
    bias = nc.dram_tensor("bias", [128, 640], F32, kind="ExternalInput")
    out = nc.dram_tensor("out", [NS, 64], F32, kind="ExternalOutput")

    with TileContext(nc) as tc:
        with (
            tc.tile_pool(name="const", bufs=1) as cpool,
            tc.tile_pool(name="inp", bufs=bufs_in) as in_pool,
            tc.tile_pool(name="tsb", bufs=bufs_t) as t_pool,
            tc.tile_pool(name="outp", bufs=bufs_out) as out_pool,
            tc.tile_pool(name="ptp", bufs=bufs_ptp, space="PSUM") as ptp_pool,
            tc.tile_pool(name="pmm", bufs=bufs_pmm, space="PSUM") as pmm_pool,
        ):
            ident = cpool.tile([128, 128], F16)
            make_identity(nc, ident[:])
            ws_sb = cpool.tile([120, 320], F16)
            nc.sync.dma_start(out=ws_sb[:], in_=ws[:])
            bias2_sb = cpool.tile([128, 640], F32)
            nc.sync.dma_start(out=bias2_sb[:], in_=bias[:])
            bias_sb = bias2_sb[:, :320]

            for base, gc in _tile_plan(gc_main):
                in_tile = in_pool.tile([128, 120 * gc_main], F16, tag="in")
                nc.sync.dma_start(
                    out=in_tile[:, : 120 * gc],
                    in_=p24[base : base + CHUNK * gc, :].rearrange(
                        "(p c t) s -> p (c t s)", p=128, c=gc, t=TB
                    ),
                )
                out_sb = out_pool.tile([128, 320 * gc_main], F32, tag="out")
                # Chunks processed in pairs: two transposes share one PSUM
                # tile / one ACT copy; two matmuls land in one two-bank PSUM
                # tile (each within its own bank) so one DVE tensor_add
                # evacuates + biases both.
                for c0 in range(0, gc, 2):
                    pair = min(2, gc - c0)
                    t_psum = ptp_pool.tile([120, 256], F16, tag="tp")
                    for k in range(pair):
                        nc.tensor.transpose(
                            t_psum[:, 128 * k : 128 * k + 128],
                            in_tile[:, 120 * (c0 + k) : 120 * (c0 + k) + 120],
                            ident[:],
                        )
                    t_sb = t_pool.tile([120, 256], F16, tag="t")
                    nc.scalar.copy(
                        out=t_sb[:, : 128 * pair], in_=t_psum[:, : 128 * pair]
                    )
                    mm_psum = pmm_pool.tile([128, 1024], F32, tag="mm")
                    for k in range(pair):
                        nc.tensor.matmul(
                            mm_psum[:, 512 * k : 512 * k + 320],
                            t_sb[:, 128 * k : 128 * k + 128],
                            ws_sb[:],
                            start=True,
                            stop=True,
                        )
                    if pair == 2:
                        src = mm_psum[:].rearrange("p (k n) -> p k n", k=2)[:, :, :320]
                        nc.vector.tensor_add(
                            out=out_sb[:, 320 * c0 : 320 * c0 + 640].rearrange(
                                "p (k n) -> p k n", k=2
                            ),
                            in0=src,
                            in1=bias2_sb[:].rearrange("p (k n) -> p k n", k=2)[
                                :, :, :320
                            ],
                        )
                    else:
                        nc.vector.tensor_add(
                            out=out_sb[:, 320 * c0 : 320 * c0 + 320],
                            in0=mm_psum[:, :320],
                            in1=bias_sb,
                        )
                nc.sync.dma_start(
                    out=out[base : base + CHUNK * gc, :].rearrange(
                        "(p c t) h -> p (c t h)", p=128, c=gc, t=TB
                    ),
                    in_=out_sb[:, : 320 * gc],
                )
    nc.compile()
    return nc


def _get_nc():
    global _nc_cache
    if _nc_cache is None:
        _nc_cache = _build()
    return _nc_cache


def _pack24(vs: np.ndarray) -> np.ndarray:
    hi = vs.astype(np.float16)
    lo = (vs - hi.astype(np.float32)).astype(np.float16)
    p = np.empty((vs.shape[0], 24), dtype=np.float16)
    p[:, 0::3] = hi
    p[:, 1::3] = lo
    p[:, 2::3] = hi
    return p


def _make_ws(W: np.ndarray) -> np.ndarray:
    w_hi = W.astype(np.float16)
    w_lo = (W - w_hi.astype(np.float32)).astype(np.float16)
    ws = np.zeros((120, 320), dtype=np.float16)
    for t in range(TB):
        for i in range(8):
            ws[24 * t + 3 * i + 0, 64 * t : 64 * t + 64] = w_hi[i]
            ws[24 * t + 3 * i + 1, 64 * t : 64 * t + 64] = w_hi[i]
            ws[24 * t + 3 * i + 2, 64 * t : 64 * t + 64] = w_lo[i]
    return ws


def kernel(vs: np.ndarray, W: np.ndarray, b: np.ndarray, _trace=False):
    vs = np.asarray(vs, dtype=np.float32)
    W = np.asarray(W, dtype=np.float32)
    b = np.asarray(b, dtype=np.float32)

    nc = _get_nc()

    ws = _make_ws(W)
    bsum = b.sum(axis=0, dtype=np.float32)
    bias = np.broadcast_to(np.tile(bsum, 2 * TB), (128, 640)).copy()

    p24 = _pack24(vs.reshape(B, 8))
    in_maps = [
        {"p24": np.ascontiguousarray(p24[k * NS : (k + 1) * NS]), "ws": ws,
         "bias": bias}
        for k in range(NCORES)
    ]

    res = run_bass_kernel_spmd(nc, in_maps, core_ids=list(range(NCORES)))
    out = np.concatenate([r["out"] for r in res.results], axis=0)
    if _trace:
        kernel.last_result = res
    return out


# revision 12
# speedup vs baseline: 1.2183x; 1.0360x over previous
"""GNN message-passing kernel for Trainium2 (8 NeuronCores, data-parallel).

Computes msg = vs @ W + b.sum(0) for vs [2M, 8] f32, W/b [8, 64] f32.

Strategy:
  - Shard vs rows 8 ways (250k rows/core); W/b replicated (no gradients here,
    forward only).
  - Precision: fp16 hi/lo split, 3 cross terms (hi*hi, lo*hi, hi*lo) gives
    fp32-grade accuracy while running the PE at 1 cycle/row (fp32 matmul is
    4 cycles/row).
  - Per 640-node chunk: the host-packed [B, 24] fp16 tensor is DMA'd in a
    (p (c t s)) layout, PE-transposed to put the 24-value groups on
    partitions (K=120 = 5 t-blocks of 24), then one matmul against a
    block-diagonal [120, 320] weight matrix produces out[p, 64t+h] =
    msg[node(p, t), h] — giving fully contiguous per-partition output DMA.
  - Bias is folded into the PSUM->SBUF evacuation (DVE tensor_add).
"""

import numpy as np
import concourse.bacc as bacc
import concourse.mybir as mybir
from concourse.tile import TileContext
from concourse.masks import make_identity
from concourse.bass_utils import run_bass_kernel_spmd

F32 = mybir.dt.float32
F16 = mybir.dt.float16

B = 2_000_000
NCORES = 8
NS = B // NCORES          # 250_000 nodes per core
TB = 5                    # t-blocks per matmul
CHUNK = 128 * TB          # 640 nodes per chunk
GC_MAIN = 16              # chunks per tile in the main loop


def _tile_plan(gc_main):
    # Full chunks per core: 390 regular + 1 overlap tile re-covering the
    # final 640 nodes (overlap region written twice with identical values).
    n_full = 390
    tiles = [(i * CHUNK * gc_main, gc_main) for i in range(n_full // gc_main)]
    rem = n_full % gc_main
    if rem:
        tiles.append(((n_full - rem) * CHUNK, rem))
    tiles.append((NS - CHUNK, 1))
    return tiles


_nc_cache = None


def _build(gc_main=GC_MAIN, bufs_in=6, bufs_t=8, bufs_out=6, bufs_ptp=2,
           bufs_pmm=3):
    nc = bacc.Bacc()
    p24 = nc.dram_tensor("p24", [NS, 24], F16, kind="ExternalInput")
    ws = nc.dram_tensor("ws", [120, 320], F16, kind="ExternalInput")
    bias = nc.dram_tensor("bias", [128, 640], F32, kind="ExternalInput")
    out = nc.dram_tensor("out", [NS, 64], F32, kind="ExternalOutput")

    with TileContext(nc) as tc:
        with (
            tc.tile_pool(name="const", bufs=1) as cpool,
            tc.tile_pool(name="inp", bufs=bufs_in) as in_pool,
            tc.tile_pool(name="tsb", bufs=bufs_t) as t_pool,
            tc.tile_pool(name="outp", bufs=bufs_out) as out_pool,
            tc.tile_pool(name="ptp", bufs=bufs_ptp, space="PSUM") as ptp_pool,
            tc.tile_pool(name="pmm", bufs=bufs_pmm, space="PSUM") as pmm_pool,
        ):
            ident = cpool.tile([128, 128], F16)
            make_identity(nc, ident[:])
            ws_sb = cpool.tile([120, 320], F16)
            nc.sync.dma_start(out=ws_sb[:], in_=ws[:])
            bias2_sb = cpool.tile([128, 640], F32)
            nc.sync.dma_start(out=bias2_sb[:], in_=bias[:])
            bias_sb = bias2_sb[:, :320]

            for base, gc in _tile_plan(gc_main):
                in_tile = in_pool.tile([128, 120 * gc_main], F16, tag="in")
                in_ap = p24[base : base + CHUNK * gc, :].rearrange(
                    "(p c t) s -> p (c t s)", p=128, c=gc, t=TB
                )
                # Split big tiles' I/O in halves so DMA and compute overlap at
                # half-tile granularity.
                h = (gc // 2) * 120 if gc == gc_main else gc * 120
                for lo in range(0, gc * 120, h):
                    nc.sync.dma_start(
                        out=in_tile[:, lo : lo + h], in_=in_ap[:, lo : lo + h]
                    )
                out_sb = out_pool.tile([128, 320 * gc_main], F32, tag="out")
                # Chunks processed in pairs: two transposes share one PSUM
                # tile / one ACT copy; two matmuls land in one two-bank PSUM
                # tile (each within its own bank) so one DVE tensor_add
                # evacuates + biases both.
                for c0 in range(0, gc, 2):
                    pair = min(2, gc - c0)
                    t_psum = ptp_pool.tile([120, 256], F16, tag="tp")
                    for k in range(pair):
                        nc.tensor.transpose(
                            t_psum[:, 128 * k : 128 * k + 128],
                            in_tile[:, 120 * (c0 + k) : 120 * (c0 + k) + 120],
                            ident[:],
                        )
                    t_sb = t_pool.tile([120, 256], F16, tag="t")
                    nc.scalar.copy(
                        out=t_sb[:, : 128 * pair], in_=t_psum[:, : 128 * pair]
                    )
                    mm_psum = pmm_pool.tile([128, 1024], F32, tag="mm")
                    for k in range(pair):
                        nc.tensor.matmul(
                            mm_psum[:, 512 * k : 512 * k + 320],
                            t_sb[:, 128 * k : 128 * k + 128],
                            ws_sb[:],
                            start=True,
                            stop=True,
                        )
                    if pair == 2:
                        src = mm_psum[:].rearrange("p (k n) -> p k n", k=2)[:, :, :320]
                        nc.vector.tensor_add(
                            out=out_sb[:, 320 * c0 : 320 * c0 + 640].rearrange(
                                "p (k n) -> p k n", k=2
                            ),
                            in0=src,
                            in1=bias2_sb[:].rearrange("p (k n) -> p k n", k=2)[
                                :, :, :320
                            ],
                        )
                    else:
                        nc.vector.tensor_add(
                            out=out_sb[:, 320 * c0 : 320 * c0 + 320],
                            in0=mm_psum[:, :320],
                            in1=bias_sb,
                        )
                out_ap = out[base : base + CHUNK * gc, :].rearrange(
                    "(p c t) h -> p (c t h)", p=128, c=gc, t=TB
                )
                ho = (gc // 2) * 320 if gc == gc_main else gc * 320
                for lo in range(0, gc * 320, ho):
                    nc.sync.dma_start(
                        out=out_ap[:, lo : lo + ho], in_=out_sb[:, lo : lo + ho]
                    )
    nc.compile()
    return nc


def _get_nc():
    global _nc_cache
    if _nc_cache is None:
        _nc_cache = _build()
    return _nc_cache


def _pack24(vs: np.ndarray) -> np.ndarray:
    hi = vs.astype(np.float16)
    lo = (vs - hi.astype(np.float32)).astype(np.float16)
    p = np.empty((vs.shape[0], 24), dtype=np.float16)
    p[:, 0::3] = hi
    p[:, 1::3] = lo
    p[:, 2::3] = hi
    return p


def _make_ws(W: np.ndarray) -> np.ndarray:
    w_hi = W.astype(np.float16)
    w_lo = (W - w_hi.astype(np.float32)).astype(np.float16)
    ws = np.zeros((120, 320), dtype=np.float16)
    for t in range(TB):
        for i in range(8):
            ws[24 * t + 3 * i + 0, 64 * t : 64 * t + 64] = w_hi[i]
            ws[24 * t + 3 * i + 1, 64 * t : 64 * t + 64] = w_hi[i]
            ws[24 * t + 3 * i + 2, 64 * t : 64 * t + 64] = w_lo[i]
    return ws


def kernel(vs: np.ndarray, W: np.ndarray, b: np.ndarray, _trace=False):
    vs = np.asarray(vs, dtype=np.float32)
    W = np.asarray(W, dtype=np.float32)
    b = np.asarray(b, dtype=np.float32)

    nc = _get_nc()

    ws = _make_ws(W)
    bsum = b.sum(axis=0, dtype=np.float32)
    bias = np.broadcast_to(np.tile(bsum, 2 * TB), (128, 640)).copy()

    p24 = _pack24(vs.reshape(B, 8))
    in_maps = [
        {"p24": np.ascontiguousarray(p24[k * NS : (k + 1) * NS]), "ws": ws,
         "bias": bias}
        for k in range(NCORES)
    ]

    res = run_bass_kernel_spmd(nc, in_maps, core_ids=list(range(NCORES)))
    out = np.concatenate([r["out"] for r in res.results], axis=0)
    if _trace:
        kernel.last_result = res
    return out
